# revision 15
# baseline (speedup 1.0000x reference)
"""MinGRU Trainium2 kernel.

Problem (hardcoded): x [N=8, T=4096, C=512] f32, W [512, 1024] f32, b [1024] f32.
  y = x @ W + b; hidden, gate = y[..., :512], y[..., 512:]
  a_t = sigmoid(-gate)            (= 1 - z)
  b_t = sigmoid(gate) * g(hidden),  g(v) = v + 0.5 (v>=0) else sigmoid(v)
  h_t = a_t * h_{t-1} + b_t  (h_{-1} = 0)   -> out [N, T, 512], last [N, 1, 512]

The reference computes this scan in log space for stability; in linear space
the recurrence is identical and well-conditioned (a_t in (0,1), g bounded),
so we run it directly with fp32 state.

Sharding: data-parallel over batch N across the 8 cores (1 sequence each).
Layout trick: host passes x[n] transposed ([C, T] contiguous) so the matmul
W.T @ x.T produces y^T with channels on partitions and T along the free
dimension; the whole T-scan then maps to the DVE's TensorTensorScanArith
instruction (state = a[:,t] * state + b[:,t] per partition lane).

g(v) identity used on-device: g(v) = max(sigmoid(v), v + 0.5)  (exact: the
two branches cross at v=0 and v+0.5-sigmoid(v) is monotone increasing).

Per (pair p of 128 channels, T-block of 1024):
  psum_hid, psum_gate = W.T @ xT           (fp32r matmuls, full PE rate)
  z = sigmoid(gate + bg)        ACT
  a = sigmoid(-gate - bg)       ACT
  s = sigmoid(hid + bh)         ACT
  g = max(hid + (bh+.5), s)     DVE scalar_tensor_tensor
  bt = z * g                    DVE
  h = scan(a, bt, carry)        DVE
DMAs are batched: one x load / one h store per T-block (all 4 k-tiles /
pairs in a single 3D access pattern), W and biases in one load each.
"""

import numpy as np

N, T, C, H = 8, 4096, 512, 512
TB = 1024           # T-block
NB = T // TB        # 4 blocks
KT = C // 128       # 4 contraction tiles
PAIRS = H // 128    # 4 hidden/gate pairs

_CACHE = {}


def _build():
    import concourse.bacc as bacc
    import concourse.tile as tile
    import concourse.mybir as mybir

    f32 = mybir.dt.float32
    f32r = mybir.dt.float32r
    Alu = mybir.AluOpType
    Sigmoid = mybir.ActivationFunctionType.Sigmoid

    nc = bacc.Bacc("TRN2", target_bir_lowering=False, debug=False,
                   enable_asserts=False, num_devices=N)

    xT_d = nc.dram_tensor("xT", [C, T], f32r, kind="ExternalInput")
    W_d = nc.dram_tensor("W", [C, 2 * H], f32r, kind="ExternalInput")
    # biases [128, 16]: cols 0-3 b_gate(pair), 4-7 -b_gate, 8-11 b_hid, 12-15 b_hid+0.5
    b_d = nc.dram_tensor("biases", [128, 16], f32, kind="ExternalInput")
    hT_d = nc.dram_tensor("hT", [H, T], f32, kind="ExternalOutput")

    # 3D views for batched DMA (partition dim first).
    # xT[c, t] with c = k*128 + q  ->  [q, k, t]
    xT_v = xT_d.ap().rearrange("(k q) t -> q k t", k=KT)
    # W[c, j] with c = k*128 + q   ->  [q, k, j]
    W_v = W_d.ap().rearrange("(k q) j -> q k j", k=KT)
    # hT[c, t] with c = p*128 + q  ->  [q, p, t]
    hT_v = hT_d.ap().rearrange("(p q) t -> q p t", p=PAIRS)

    with tile.TileContext(nc) as tc:
        with (
            tc.tile_pool(name="const", bufs=1) as const_pool,
            tc.tile_pool(name="x", bufs=2) as x_pool,
            tc.tile_pool(name="ew", bufs=5) as ew_pool,
            tc.tile_pool(name="h", bufs=2) as h_pool,
            tc.tile_pool(name="ps", bufs=1, space="PSUM") as ps_pool,
        ):
            bias_t = const_pool.tile([128, 16], f32, tag="bias")
            # W/bias dispatch on the (idle at t=0) ACT HWDGE queue so they
            # don't serialize behind the x dispatches on the Sync queue
            nc.scalar.dma_start(bias_t[:], b_d[:])

            # W as one tile per k so a matmul only waits for the k-chunk it reads
            w_k = []
            for k in range(KT):
                wk = const_pool.tile([128, 2 * H], f32r, tag=f"w{k}")
                nc.scalar.dma_start(wk[:], W_v[:, k])
                w_k.append(wk)

            def w_sl(k, m):
                return w_k[k][:, m * 128:(m + 1) * 128]

            prev_h = [None] * PAIRS
            # smaller first blocks prime the pipeline sooner
            blocks = [(0, 256), (256, 256), (512, 512), (1024, 1024),
                      (2048, 1024), (3072, 1024)]
            for i, (t0, tb) in enumerate(blocks):
                last = i == len(blocks) - 1
                # x as one tile per k: a matmul waits only for its own k-chunk
                xk = []
                for k in range(KT):
                    xkt = x_pool.tile([128, tb], f32r, tag=f"x{k}")
                    nc.sync.dma_start(xkt[:], xT_v[:, k, t0:t0 + tb])
                    xk.append(xkt)

                h_big = h_pool.tile([128, PAIRS * tb], f32, tag="h")

                for p in range(PAIRS):
                    ph = ps_pool.tile([128, tb], f32, tag=f"hid{p % 2}")
                    pg = ps_pool.tile([128, tb], f32, tag=f"gate{p % 2}")
                    for (pm, m) in ((ph, p), (pg, PAIRS + p)):
                        for o in range(0, tb, 512):
                            n = min(512, tb - o)
                            osl = slice(o, o + n)
                            for k in range(KT):
                                nc.tensor.matmul(
                                    pm[:, osl], w_sl(k, m),
                                    xk[k][:, osl],
                                    start=(k == 0), stop=(k == KT - 1))

                    # s first: it gates the DVE chain (g -> bt -> scan)
                    s = ew_pool.tile([128, tb], f32, tag="s")
                    nc.scalar.activation(s[:], ph[:], Sigmoid,
                                         bias=bias_t[:, 8 + p:9 + p], scale=1.0)
                    z = ew_pool.tile([128, tb], f32, tag="z")
                    nc.scalar.activation(z[:], pg[:], Sigmoid,
                                         bias=bias_t[:, p:p + 1], scale=1.0)
                    a = ew_pool.tile([128, tb], f32, tag="a")
                    nc.scalar.activation(a[:], pg[:], Sigmoid,
                                         bias=bias_t[:, 4 + p:5 + p], scale=-1.0)
                    g = ew_pool.tile([128, tb], f32, tag="g")
                    nc.vector.scalar_tensor_tensor(
                        g[:], ph[:], bias_t[:, 12 + p:13 + p], s[:],
                        op0=Alu.add, op1=Alu.max)
                    bt = ew_pool.tile([128, tb], f32, tag="bt")
                    nc.vector.tensor_tensor(bt[:], z[:], g[:], op=Alu.mult)

                    hsl = h_big[:, p * tb:(p + 1) * tb]
                    init = 0.0 if i == 0 else prev_h[p]
                    nc.vector.tensor_tensor_scan(hsl, a[:], bt[:], initial=init,
                                                 op0=Alu.mult, op1=Alu.add)
                    prev_h[p] = h_big[:, (p + 1) * tb - 1:(p + 1) * tb]

                    if last:
                        # last block: store per pair (short drain tail)
                        nc.sync.dma_start(hT_v[:, p, t0:t0 + tb], hsl)

                if not last:
                    # one DMA for all 4 pairs of this T-block
                    nc.sync.dma_start(
                        hT_v[:, :, t0:t0 + tb],
                        h_big[:].rearrange("q (p t) -> q p t", p=PAIRS))

    nc.compile()
    return nc


def _get_nc():
    if "nc" not in _CACHE:
        _CACHE["nc"] = _build()
    return _CACHE["nc"]


def _make_in_maps(x, W, b):
    x = np.ascontiguousarray(x, dtype=np.float32)
    W = np.ascontiguousarray(W, dtype=np.float32)
    b = np.ascontiguousarray(b, dtype=np.float32)

    biases = np.empty((128, 16), dtype=np.float32)
    for p in range(PAIRS):
        bg = b[H + p * 128: H + (p + 1) * 128]
        bh = b[p * 128:(p + 1) * 128]
        biases[:, p] = bg
        biases[:, 4 + p] = -bg
        biases[:, 8 + p] = bh
        biases[:, 12 + p] = bh + 0.5

    in_maps = []
    for n in range(N):
        in_maps.append({
            "xT": np.ascontiguousarray(x[n].T),
            "W": W,
            "biases": biases,
        })
    return in_maps


def _run(x, W, b, trace=False):
    from concourse.bass_utils import run_bass_kernel_spmd
    nc = _get_nc()
    res = run_bass_kernel_spmd(nc, _make_in_maps(x, W, b),
                               core_ids=list(range(N)), trace=trace)
    out = np.empty((N, T, H), dtype=np.float32)
    for n in range(N):
        out[n] = res.results[n]["hT"].T
    last = np.ascontiguousarray(out[:, -1:, :])
    return (out, last), res


def kernel(x, W, b):
    (out, last), _ = _run(x, W, b, trace=False)
    return (out, last)


# revision 16
# speedup vs baseline: 1.0358x; 1.0358x over previous
"""MinGRU Trainium2 kernel.

Problem (hardcoded): x [N=8, T=4096, C=512] f32, W [512, 1024] f32, b [1024] f32.
  y = x @ W + b; hidden, gate = y[..., :512], y[..., 512:]
  a_t = sigmoid(-gate)            (= 1 - z)
  b_t = sigmoid(gate) * g(hidden),  g(v) = v + 0.5 (v>=0) else sigmoid(v)
  h_t = a_t * h_{t-1} + b_t  (h_{-1} = 0)   -> out [N, T, 512], last [N, 1, 512]

The reference computes this scan in log space for stability; in linear space
the recurrence is identical and well-conditioned (a_t in (0,1), g bounded),
so we run it directly with fp32 state.

Sharding: data-parallel over batch N across the 8 cores (1 sequence each).
Layout trick: host passes x[n] transposed ([C, T] contiguous) so the matmul
W.T @ x.T produces y^T with channels on partitions and T along the free
dimension; the whole T-scan then maps to the DVE's TensorTensorScanArith
instruction (state = a[:,t] * state + b[:,t] per partition lane).

g(v) identity used on-device: g(v) = max(sigmoid(v), v + 0.5)  (exact: the
two branches cross at v=0 and v+0.5-sigmoid(v) is monotone increasing).

Per (pair p of 128 channels, T-block of 1024):
  psum_hid, psum_gate = W.T @ xT           (fp32r matmuls, full PE rate)
  z = sigmoid(gate + bg)        ACT
  a = sigmoid(-gate - bg)       ACT
  s = sigmoid(hid + bh)         ACT
  g = max(hid + (bh+.5), s)     DVE scalar_tensor_tensor
  bt = z * g                    DVE
  h = scan(a, bt, carry)        DVE
DMAs are batched: one x load / one h store per T-block (all 4 k-tiles /
pairs in a single 3D access pattern), W and biases in one load each.
"""

import numpy as np

N, T, C, H = 8, 4096, 512, 512
TB = 1024           # T-block
NB = T // TB        # 4 blocks
KT = C // 128       # 4 contraction tiles
PAIRS = H // 128    # 4 hidden/gate pairs

_CACHE = {}


def _build():
    import concourse.bacc as bacc
    import concourse.tile as tile
    import concourse.mybir as mybir

    f32 = mybir.dt.float32
    f32r = mybir.dt.float32r
    Alu = mybir.AluOpType
    Sigmoid = mybir.ActivationFunctionType.Sigmoid

    nc = bacc.Bacc("TRN2", target_bir_lowering=False, debug=False,
                   enable_asserts=False, num_devices=N)

    xT_d = nc.dram_tensor("xT", [C, T], f32r, kind="ExternalInput")
    W_d = nc.dram_tensor("W", [C, 2 * H], f32r, kind="ExternalInput")
    # biases [128, 16]: cols 0-3 b_gate(pair), 4-7 -b_gate, 8-11 b_hid, 12-15 b_hid+0.5
    b_d = nc.dram_tensor("biases", [128, 16], f32, kind="ExternalInput")
    hT_d = nc.dram_tensor("hT", [H, T], f32, kind="ExternalOutput")

    # 3D views for batched DMA (partition dim first).
    # xT[c, t] with c = k*128 + q  ->  [q, k, t]
    xT_v = xT_d.ap().rearrange("(k q) t -> q k t", k=KT)
    # W[c, j] with c = k*128 + q   ->  [q, k, j]
    W_v = W_d.ap().rearrange("(k q) j -> q k j", k=KT)
    # hT[c, t] with c = p*128 + q  ->  [q, p, t]
    hT_v = hT_d.ap().rearrange("(p q) t -> q p t", p=PAIRS)

    with tile.TileContext(nc) as tc:
        with (
            tc.tile_pool(name="const", bufs=1) as const_pool,
            tc.tile_pool(name="x", bufs=2) as x_pool,
            tc.tile_pool(name="ew", bufs=5) as ew_pool,
            tc.tile_pool(name="h", bufs=2) as h_pool,
            tc.tile_pool(name="ps", bufs=1, space="PSUM") as ps_pool,
        ):
            bias_t = const_pool.tile([128, 16], f32, tag="bias")
            # W/bias dispatch on the (idle at t=0) ACT HWDGE queue so they
            # don't serialize behind the x dispatches on the Sync queue
            nc.scalar.dma_start(bias_t[:], b_d[:])

            # W as one tile per k so a matmul only waits for the k-chunk it reads
            w_k = []
            for k in range(KT):
                wk = const_pool.tile([128, 2 * H], f32r, tag=f"w{k}")
                nc.scalar.dma_start(wk[:], W_v[:, k])
                w_k.append(wk)

            def w_sl(k, m):
                return w_k[k][:, m * 128:(m + 1) * 128]

            prev_h = [None] * PAIRS
            # smaller first/last blocks: faster pipeline prime, shorter drain
            blocks = [(0, 512), (512, 1024), (1536, 1024), (2560, 1024),
                      (3584, 512)]
            for i, (t0, tb) in enumerate(blocks):
                last = i == len(blocks) - 1
                # x as one tile per k: a matmul waits only for its own k-chunk
                xk = []
                for k in range(KT):
                    xkt = x_pool.tile([128, tb], f32r, tag=f"x{k}")
                    nc.sync.dma_start(xkt[:], xT_v[:, k, t0:t0 + tb])
                    xk.append(xkt)

                h_big = h_pool.tile([128, PAIRS * tb], f32, tag="h")

                for p in range(PAIRS):
                    ph = ps_pool.tile([128, tb], f32, tag=f"hid{p % 2}")
                    pg = ps_pool.tile([128, tb], f32, tag=f"gate{p % 2}")
                    for (pm, m) in ((ph, p), (pg, PAIRS + p)):
                        for o in range(0, tb, 512):
                            n = min(512, tb - o)
                            osl = slice(o, o + n)
                            for k in range(KT):
                                nc.tensor.matmul(
                                    pm[:, osl], w_sl(k, m),
                                    xk[k][:, osl],
                                    start=(k == 0), stop=(k == KT - 1))

                    # s first: it gates the DVE chain (g -> bt -> scan)
                    s = ew_pool.tile([128, tb], f32, tag="s")
                    nc.scalar.activation(s[:], ph[:], Sigmoid,
                                         bias=bias_t[:, 8 + p:9 + p], scale=1.0)
                    z = ew_pool.tile([128, tb], f32, tag="z")
                    nc.scalar.activation(z[:], pg[:], Sigmoid,
                                         bias=bias_t[:, p:p + 1], scale=1.0)
                    a = ew_pool.tile([128, tb], f32, tag="a")
                    nc.scalar.activation(a[:], pg[:], Sigmoid,
                                         bias=bias_t[:, 4 + p:5 + p], scale=-1.0)
                    g = ew_pool.tile([128, tb], f32, tag="g")
                    nc.vector.scalar_tensor_tensor(
                        g[:], ph[:], bias_t[:, 12 + p:13 + p], s[:],
                        op0=Alu.add, op1=Alu.max)
                    bt = ew_pool.tile([128, tb], f32, tag="bt")
                    nc.vector.tensor_tensor(bt[:], z[:], g[:], op=Alu.mult)

                    hsl = h_big[:, p * tb:(p + 1) * tb]
                    init = 0.0 if i == 0 else prev_h[p]
                    nc.vector.tensor_tensor_scan(hsl, a[:], bt[:], initial=init,
                                                 op0=Alu.mult, op1=Alu.add)
                    prev_h[p] = h_big[:, (p + 1) * tb - 1:(p + 1) * tb]

                    if last:
                        # last block: store per pair (short drain tail)
                        nc.sync.dma_start(hT_v[:, p, t0:t0 + tb], hsl)

                if not last:
                    # one DMA for all 4 pairs of this T-block
                    nc.sync.dma_start(
                        hT_v[:, :, t0:t0 + tb],
                        h_big[:].rearrange("q (p t) -> q p t", p=PAIRS))

    nc.compile()
    return nc


def _get_nc():
    if "nc" not in _CACHE:
        _CACHE["nc"] = _build()
    return _CACHE["nc"]


def _make_in_maps(x, W, b):
    x = np.ascontiguousarray(x, dtype=np.float32)
    W = np.ascontiguousarray(W, dtype=np.float32)
    b = np.ascontiguousarray(b, dtype=np.float32)

    biases = np.empty((128, 16), dtype=np.float32)
    for p in range(PAIRS):
        bg = b[H + p * 128: H + (p + 1) * 128]
        bh = b[p * 128:(p + 1) * 128]
        biases[:, p] = bg
        biases[:, 4 + p] = -bg
        biases[:, 8 + p] = bh
        biases[:, 12 + p] = bh + 0.5

    in_maps = []
    for n in range(N):
        in_maps.append({
            "xT": np.ascontiguousarray(x[n].T),
            "W": W,
            "biases": biases,
        })
    return in_maps


def _run(x, W, b, trace=False):
    from concourse.bass_utils import run_bass_kernel_spmd
    nc = _get_nc()
    res = run_bass_kernel_spmd(nc, _make_in_maps(x, W, b),
                               core_ids=list(range(N)), trace=trace)
    out = np.empty((N, T, H), dtype=np.float32)
    for n in range(N):
        out[n] = res.results[n]["hT"].T
    last = np.ascontiguousarray(out[:, -1:, :])
    return (out, last), res


def kernel(x, W, b):
    (out, last), _ = _run(x, W, b, trace=False)
    return (out, last)
